# revision 13
# baseline (speedup 1.0000x reference)
"""Multi-head self-attention (B=2, S=2048, D=1024, H=16) on 8 Trainium2 cores.

Sharding: Megatron-style tensor parallelism on the head dimension.
Each core owns 2 heads (128 of the 1024 model dims):
  - Wq/Wk/Wv column-sharded: core c computes Q/K/V for dims [c*128,(c+1)*128)
  - attention for its 2 heads over both batches
  - Wo row-sharded: core c produces a partial output [4096, 1024] (bf16)
  - host sums the 8 partials and adds bo.

All matmul operands are bf16 (fp32 PSUM accumulate). Device schedule is
software-pipelined so the PE never idles long enough for the HAM clock
gate to re-throttle it to 1.2 GHz, and so the scalar engine (exp, the
per-iteration critical path in attention) always has score tiles ready:

  1. K/V projections for batch 0 (PE-dense warmup)
  2. Q projection for batch 0 / chunk 0
  3. attention blocks, with the remaining projection work (b0 q chunks,
     batch-1 k/v, batch-1 q) interleaved between score/PV groups as PE
     filler while the scalar engine chews through exp. Forced-drain
     checkpoints before each block guarantee its inputs are emitted
     earlier in the (in-order) engine queues.

Attention layouts (per core):
  qT/kT: [128(out-dim), 4096(token)]; head h lives on partitions
         [h*64,(h+1)*64) so the two heads' score matmuls land on
         different row-tiles of the PE array and stream concurrently.
  vtk:   token-major k-tiles [128(token), 132] = 2x [head(64)|ones|pad];
         the ones column makes the PV matmul emit the softmax
         normalizer as output row 64.
  scores are computed transposed: sT[k, q], so the softmax sum reduces
  over the PARTITION dim via the ones row. Scores for 2 consecutive
  k-tiles x 2 heads share one 4-bank PSUM tile so a single 2048-wide
  exp covers them (amortizes the ~350-cycle ACTIVATE fixed cost).
  exp needs no max subtraction: scores*0.125 are ~N(0,1) here.
"""

import os
import numpy as np
import ml_dtypes
from contextlib import ExitStack

import concourse.bass as bass
import concourse.tile as tile
from concourse import bacc, mybir
from concourse.bass_utils import run_bass_kernel_spmd
from concourse.masks import make_identity

B, S, D = 2, 2048, 1024
H, DH = 16, 64
T = B * S                  # 4096 tokens total
N_CORES = 8
OPC = D // N_CORES         # 128 out dims per core
HPC = H // N_CORES         # 2 heads per core
NI = D // 128              # 8 contraction chunks of 128
TCH = 512                  # projection token chunk
NTCH = T // TCH            # 8
QCH = 512                  # attention q chunk
NQCH = S // QCH            # 4 per batch
NKT = S // 128             # 16 key tiles per batch
NG = NKT // 2              # 8 score groups (2 k-tiles each) per q chunk
HW = DH + 2                # 66 cols per head in the v tile (data|ones|pad)
VW = HPC * HW              # 132

F32 = mybir.dt.float32
F32R = mybir.dt.float32r
BF16 = mybir.dt.bfloat16
EXP = mybir.ActivationFunctionType.Exp
MM_NP = ml_dtypes.bfloat16


def _mha_kernel(tc, y, xT, wq, wk, wv, woT, bq, bk, bv):
    with ExitStack() as ctx:
        _mha_kernel_inner(ctx, tc, y, xT, wq, wk, wv, woT, bq, bk, bv)


def _mha_kernel_inner(ctx: ExitStack, tc, y, xT, wq, wk, wv, woT, bq, bk, bv):
    nc = tc.nc
    pers = ctx.enter_context(tc.tile_pool(name="pers", bufs=1))

    qT = pers.tile([128, T], BF16, tag="qT")
    kT = pers.tile([128, T], BF16, tag="kT")
    vtk = pers.tile([128, B * NKT, VW], F32R, tag="vtk")
    xfull = pers.tile([128, NI, T], BF16, tag="xfull")
    wq_sb = pers.tile([128, NI, OPC], BF16, tag="wq")
    wk_sb = pers.tile([128, NI, OPC], BF16, tag="wk")
    wv_sb = pers.tile([128, NI, OPC], BF16, tag="wv")
    woT_sb = pers.tile([128, D], BF16, tag="wo")
    bq_sb = pers.tile([128, 1], F32, tag="bq")
    bk_sb = pers.tile([128, 1], F32, tag="bk")
    bv_sb = pers.tile([128, 1], F32, tag="bv")
    ident = pers.tile([128, 128], F32, tag="ident")

    # identity build first: it shares the gpsimd queue with the weight
    # DMAs and everything PE-transpose waits on it
    make_identity(nc, ident)
    # one DMA per weight tensor (dma_start costs ~0.6us of issue time on
    # the queueing engine; 28 per-chunk DMAs serialized the old lead-in)
    nc.gpsimd.dma_start(wk_sb[:, :, :], wk[:, :, :])
    nc.gpsimd.dma_start(wv_sb[:, :, :], wv[:, :, :])
    nc.gpsimd.dma_start(wq_sb[:, :, :], wq[:, :, :])
    nc.gpsimd.dma_start(woT_sb, woT)
    nc.gpsimd.dma_start(bk_sb, bk)
    nc.gpsimd.dma_start(bv_sb, bv)
    nc.gpsimd.dma_start(bq_sb, bq)
    # x: one token-major DMA per 512-token chunk (all 8 contraction
    # slices in one InstDMACopy, which fans out over the 16 SDMA slots)
    for t in range(NTCH):
        src_ap = bass.AP(
            tensor=xT.tensor,
            offset=xT.offset + t * TCH,
            ap=[[T, 128], [128 * T, NI], [1, TCH]],
        )
        nc.sync.dma_start(xfull[:, :, t * TCH : (t + 1) * TCH], src_ap)
    # constant ones/pad columns of vtk (broadcast along the k-tile dim)
    onepad = pers.tile([128, 2], F32, tag="onepad")
    nc.vector.memset(onepad[:, 0:1], 1.0)
    nc.vector.memset(onepad[:, 1:2], 0.0)
    onepad_b = bass.AP(
        tensor=onepad.tensor,
        offset=onepad.offset,
        ap=[onepad.ap[0], [0, B * NKT], onepad.ap[1]],
    )
    for h in range(HPC):
        nc.vector.tensor_copy(
            vtk[:, :, h * HW + DH : h * HW + DH + 2], onepad_b
        )

    # PSUM budget (8 banks):
    #   ps_s  [128, 2x512] f32  2 banks, bufs=2 (score tiles)
    #   pvout [66, 512]    f32  1 bank,  bufs=2 (PV accumulators)
    #   gen   [128, 512]   f32  1 bank,  bufs=1 (projections, transposes)
    #   out   [128, 512]   f32  1 bank,  bufs=1 (output projection)
    psum = ctx.enter_context(tc.tile_pool(name="psum", bufs=1, space="PSUM"))
    vstage = ctx.enter_context(tc.tile_pool(name="vstage", bufs=2))
    att = ctx.enter_context(tc.tile_pool(name="att", bufs=4))
    sm = ctx.enter_context(tc.tile_pool(name="sm", bufs=3))
    yo_p = ctx.enter_context(tc.tile_pool(name="yo_p", bufs=4))

    def proj_mm(dst, w_sb, b_sb, t):
        """PE thunks for one projection over token chunk t: 8
        accumulating matmuls into the gen psum slot, then a DVE bias-add
        drain into dst (a [128, TCH] bf16 AP)."""
        ps = psum.tile([128, TCH], F32, tag="gen", bufs=1, name=f"ps_{t}")

        def mk(i):
            def f():
                nc.tensor.matmul(
                    ps,
                    w_sb[:, i, :],
                    xfull[:, i, t * TCH : (t + 1) * TCH],
                    start=(i == 0),
                    stop=(i == NI - 1),
                )
                if i == NI - 1:
                    nc.vector.tensor_scalar_add(dst, ps, b_sb)

            return f

        for i in range(NI):
            yield mk(i)

    def v_chunk(t):
        """PE thunks: V projection for chunk t, then transposes into
        vtk's token-major k-tiles."""
        vs = vstage.tile([128, TCH], F32, tag="vs", name=f"vs_{t}")
        yield from proj_mm(vs[:, :], wv_sb, bv_sb, t)

        def mk_tr(j):
            def f():
                g = (t * TCH) // 128 + j
                ps_t = psum.tile([128, 128], F32, tag="gen", bufs=1,
                                 name=f"ps_t_{t}_{j}")
                nc.tensor.transpose(ps_t, vs[:, j * 128 : (j + 1) * 128], ident)
                # both heads' 64-col slices in one strided DVE copy
                dst = bass.AP(
                    tensor=vtk.tensor,
                    offset=vtk.offset + g * VW,
                    ap=[vtk.ap[0], [HW, HPC], [1, DH]],
                )
                src_ = bass.AP(
                    tensor=ps_t.tensor,
                    offset=ps_t.offset,
                    ap=[ps_t.ap[0], [DH, HPC], [1, DH]],
                )
                nc.vector.tensor_copy(dst, src_)
            return f

        for j in range(TCH // 128):
            yield mk_tr(j)

    # ---- work stream: all projection thunks, with named checkpoints ----
    # Order: k0 q0 | v0 k1 v1 k2 v2 k3 v3 | q1 k4 v4 q2 k5 v5 q3 k6 v6
    # k7 v7 | q4 q5 q6 q7.  Attention blocks force-drain to the
    # checkpoint they need (k-chunk per score tile, v-chunk per PV tile,
    # q-chunk per block) and otherwise pull opportunistically.
    stream = []
    cp = {}

    def add(name, gen):
        stream.extend(gen)
        cp[name] = len(stream)

    add("k0", proj_mm(kT[:, 0:TCH], wk_sb, bk_sb, 0))
    add("q0", proj_mm(qT[:, 0:TCH], wq_sb, bq_sb, 0))
    add("v0", v_chunk(0))
    for t in range(1, NQCH):
        add(f"k{t}", proj_mm(kT[:, t * TCH : (t + 1) * TCH], wk_sb, bk_sb, t))
        add(f"v{t}", v_chunk(t))
    # interleave remaining q chunks among b1 k/v so every checkpoint is
    # reached by gentle pacing instead of a burst:
    # q1 k4 v4 q2 k5 v5 q3 q4 k6 v6 q5 k7 q6 v7 q7
    _q = lambda t: add(f"q{t}",
                       proj_mm(qT[:, t * TCH : (t + 1) * TCH], wq_sb, bq_sb, t))
    _k = lambda t: add(f"k{t}",
                       proj_mm(kT[:, t * TCH : (t + 1) * TCH], wk_sb, bk_sb, t))
    _v = lambda t: add(f"v{t}", v_chunk(t))
    _q(1); _k(4); _v(4); _q(2); _k(5); _v(5); _q(3); _q(4)
    _k(6); _v(6); _q(5); _k(7); _q(6); _v(7); _q(7)

    pulled = [0]

    def pull_to(level):
        level = min(int(level), len(stream))
        while pulled[0] < level:
            stream[pulled[0]]()
            pulled[0] += 1

    def drain_to(name):
        pull_to(cp[name])

    def attention_block(b, qc, tail_thunks, lvl0, lvl1):
        dlvl = lvl1 - lvl0
        q0 = b * S + qc * QCH
        at_tiles = []
        pvs = [None, None]

        def scores_group(g2):
            drain_to(f"k{b * NQCH + g2 // 4}")
            ps_s = psum.tile([128, HPC * QCH], F32, tag="ps_s", bufs=2,
                             name=f"ps_s_{b}_{qc}_{g2}")
            g = b * NKT + g2
            for h in range(HPC):
                hs = slice(h * DH, (h + 1) * DH)
                nc.tensor.matmul(
                    ps_s[:, h * QCH : (h + 1) * QCH],
                    kT[hs, g * 128 : (g + 1) * 128],
                    qT[hs, q0 : q0 + QCH],
                    start=True,
                    stop=True,
                )
            at = att.tile([128, HPC * QCH], F32R, tag="at",
                          name=f"at_{b}_{qc}_{g2}")
            nc.scalar.activation(at, ps_s, EXP, scale=0.125)
            at_tiles.append(at)

        def pv_group(g2):
            drain_to(f"v{(b * NKT + g2) // 4}")
            g = b * NKT + g2
            for h in range(HPC):
                nc.tensor.matmul(
                    pvs[h],
                    vtk[:, g, h * HW : (h + 1) * HW],
                    at_tiles[g2][:, h * QCH : (h + 1) * QCH],
                    start=(g2 == 0),
                    stop=(g2 == NKT - 1),
                )

        drain_to(f"q{b * NQCH + qc}")
        scores_group(0)
        scores_group(1)
        scores_group(2)
        pvs[0] = psum.tile([HW, QCH], F32, tag="pvout", bufs=2,
                           name=f"pv0_{b}_{qc}")
        pvs[1] = psum.tile([HW, QCH], F32, tag="pvout", bufs=2,
                           name=f"pv1_{b}_{qc}")
        for g2 in range(3, NKT):
            pv_group(g2 - 3)
            if tail_thunks:
                tail_thunks.pop(0)()
            pull_to(lvl0 + dlvl * (g2 - 2) / (NKT - 2))
            scores_group(g2)
        pv_group(NKT - 3)
        for f in tail_thunks:
            f()
        tail_thunks.clear()
        pv_group(NKT - 2)
        pv_group(NKT - 1)

        # normalize: ctx rows for head h = pv[0:64] * recip(pv[64])
        ctx_sb = sm.tile([128, QCH], BF16, tag="ctx", name=f"ctx_{b}_{qc}")
        for h in range(HPC):
            rraw = sm.tile([1, QCH], F32, tag="rraw", name=f"rraw_{b}_{qc}_{h}")
            nc.vector.tensor_copy(rraw, pvs[h][DH : DH + 1, :])
            rrow = sm.tile([1, QCH], F32, tag="rrow", name=f"rrow_{b}_{qc}_{h}")
            nc.vector.reciprocal_approx_fast(rrow, rraw)
            nrm = sm.tile([DH, QCH], F32, tag="nrm", name=f"nrm_{b}_{qc}_{h}")
            nc.gpsimd.partition_broadcast(nrm, rrow)
            nc.vector.tensor_mul(
                ctx_sb[h * DH : (h + 1) * DH, :], pvs[h][0:DH, :], nrm
            )

        # deferred out-projection: 8 (matmul, psum-drain) thunks spread
        # across the next block, on a dedicated psum bank
        new_tail = []
        yos = [yo_p.tile([128, D], F32, tag="yo", name=f"yo_{b}_{qc}_{t4}")
               for t4 in range(QCH // 128)]

        def mk_out(t4, nch):
            def f():
                ps_o = psum.tile([128, 512], F32, tag="out", bufs=1,
                                 name=f"ps_o_{b}_{qc}_{t4}_{nch}")
                nc.tensor.matmul(
                    ps_o,
                    ctx_sb[:, t4 * 128 : (t4 + 1) * 128],
                    woT_sb[:, nch * 512 : (nch + 1) * 512],
                    start=True,
                    stop=True,
                )
                nc.vector.tensor_copy(
                    yos[t4][:, nch * 512 : (nch + 1) * 512], ps_o)
                if nch == 1:
                    r0 = q0 + t4 * 128
                    nc.gpsimd.dma_start(y[r0 : r0 + 128, :], yos[t4])
            return f

        for t4 in range(QCH // 128):
            for nch in range(D // 512):
                new_tail.append(mk_out(t4, nch))
        return new_tail

    # ---- emission ----
    # linear drain schedule: block (0,0) force-drains its own needs
    # (~88 thunks); the remaining stream is spread evenly over the rest
    tail_thunks = []
    n_blocks = B * NQCH
    if os.environ.get("MHA_SERIAL"):
        pull_to(10**9)
    base = cp["v3"]
    rest = len(stream) - base
    # end-of-block targets: everything the NEXT block needs at its start
    # (its q chunk), or the linear share, whichever is larger
    targets = []
    for bi in range(n_blocks):
        lin = base + rest * bi / (n_blocks - 1)
        nxt = cp[f"q{bi + 1}"] if bi + 1 < n_blocks else len(stream)
        targets.append(max(lin, nxt))
    for bi in range(n_blocks):
        b, qc = divmod(bi, NQCH)
        lvl0 = base if bi == 0 else targets[bi - 1]
        tail_thunks = attention_block(b, qc, tail_thunks, lvl0, targets[bi])
    pull_to(10**9)
    for f in tail_thunks:
        f()


_NC_CACHE = {}


def _build_nc(repeats=1):
    if repeats in _NC_CACHE:
        return _NC_CACHE[repeats]
    nc = bacc.Bacc("TRN2", target_bir_lowering=False, debug=False, num_devices=N_CORES)
    xT = nc.dram_tensor("xT", [NI, 128, T], BF16, kind="ExternalInput").ap()
    wq = nc.dram_tensor("wq", [128, NI, OPC], BF16, kind="ExternalInput").ap()
    wk = nc.dram_tensor("wk", [128, NI, OPC], BF16, kind="ExternalInput").ap()
    wv = nc.dram_tensor("wv", [128, NI, OPC], BF16, kind="ExternalInput").ap()
    woT = nc.dram_tensor("woT", [128, D], BF16, kind="ExternalInput").ap()
    bq = nc.dram_tensor("bq", [128, 1], F32, kind="ExternalInput").ap()
    bk = nc.dram_tensor("bk", [128, 1], F32, kind="ExternalInput").ap()
    bv = nc.dram_tensor("bv", [128, 1], F32, kind="ExternalInput").ap()
    y = nc.dram_tensor("y", [T, D], F32, kind="ExternalOutput").ap()
    with tile.TileContext(nc) as tc:
        for _ in range(repeats):
            _mha_kernel(tc, y, xT, wq, wk, wv, woT, bq, bk, bv)
    nc.compile()
    _NC_CACHE[repeats] = nc
    return nc


def _prep_in_maps(inputs):
    x = np.asarray(inputs["x"], np.float32)
    Wq = np.asarray(inputs["Wq"], np.float32)
    Wk = np.asarray(inputs["Wk"], np.float32)
    Wv = np.asarray(inputs["Wv"], np.float32)
    Wo = np.asarray(inputs["Wo"], np.float32)
    bq = np.asarray(inputs["bq"], np.float32)
    bk = np.asarray(inputs["bk"], np.float32)
    bv = np.asarray(inputs["bv"], np.float32)

    xT_np = np.ascontiguousarray(x.reshape(T, D).T).reshape(NI, 128, T).astype(MM_NP)

    def _w_slice(W, c):
        # [128(p), NI, OPC]: [p, i, o] = W[c*OPC+o, i*128+p]
        A = np.ascontiguousarray(W[c * OPC : (c + 1) * OPC, :].T)  # [D, OPC]
        return np.ascontiguousarray(A.reshape(NI, 128, OPC).transpose(1, 0, 2)).astype(
            MM_NP
        )

    in_maps = []
    for c in range(N_CORES):
        sl = slice(c * OPC, (c + 1) * OPC)
        in_maps.append(
            {
                "xT": xT_np,
                "wq": _w_slice(Wq, c),
                "wk": _w_slice(Wk, c),
                "wv": _w_slice(Wv, c),
                "woT": np.ascontiguousarray(Wo[:, sl].T).astype(MM_NP),
                "bq": bq[sl].reshape(OPC, 1).copy(),
                "bk": bk[sl].reshape(OPC, 1).copy(),
                "bv": bv[sl].reshape(OPC, 1).copy(),
            }
        )
    return in_maps


def kernel(**inputs) -> np.ndarray:
    nc = _build_nc()
    in_maps = _prep_in_maps(inputs)
    res = run_bass_kernel_spmd(nc, in_maps, core_ids=list(range(N_CORES)))
    bo = np.asarray(inputs["bo"], np.float32)
    y = np.zeros((T, D), np.float32)
    for c in range(N_CORES):
        y += res.results[c]["y"].astype(np.float32)
    y = (y + bo).astype(np.float32)
    return y.reshape(B, S, D)


# revision 14
# speedup vs baseline: 1.0639x; 1.0639x over previous
"""Multi-head self-attention (B=2, S=2048, D=1024, H=16) on 8 Trainium2 cores.

Sharding: Megatron-style tensor parallelism on the head dimension.
Each core owns 2 heads (128 of the 1024 model dims):
  - Wq/Wk/Wv column-sharded: core c computes Q/K/V for dims [c*128,(c+1)*128)
  - attention for its 2 heads over both batches
  - Wo row-sharded: core c produces a partial output [4096, 1024] (bf16)
  - host sums the 8 partials and adds bo.

All matmul operands are bf16 (fp32 PSUM accumulate). Device schedule is
software-pipelined so the PE never idles long enough for the HAM clock
gate to re-throttle it to 1.2 GHz, and so the scalar engine (exp, the
per-iteration critical path in attention) always has score tiles ready:

  1. K/V projections for batch 0 (PE-dense warmup)
  2. Q projection for batch 0 / chunk 0
  3. attention blocks, with the remaining projection work (b0 q chunks,
     batch-1 k/v, batch-1 q) interleaved between score/PV groups as PE
     filler while the scalar engine chews through exp. Forced-drain
     checkpoints before each block guarantee its inputs are emitted
     earlier in the (in-order) engine queues.

Attention layouts (per core):
  qT/kT: [128(out-dim), 4096(token)]; head h lives on partitions
         [h*64,(h+1)*64) so the two heads' score matmuls land on
         different row-tiles of the PE array and stream concurrently.
  vtk:   token-major k-tiles [128(token), 132] = 2x [head(64)|ones|pad];
         the ones column makes the PV matmul emit the softmax
         normalizer as output row 64.
  scores are computed transposed: sT[k, q], so the softmax sum reduces
  over the PARTITION dim via the ones row. Scores for 2 consecutive
  k-tiles x 2 heads share one 4-bank PSUM tile so a single 2048-wide
  exp covers them (amortizes the ~350-cycle ACTIVATE fixed cost).
  exp needs no max subtraction: scores*0.125 are ~N(0,1) here.
"""

import os
import numpy as np
import ml_dtypes
from contextlib import ExitStack

import concourse.bass as bass
import concourse.tile as tile
from concourse import bacc, mybir
from concourse.bass_utils import run_bass_kernel_spmd
from concourse.masks import make_identity

B, S, D = 2, 2048, 1024
H, DH = 16, 64
T = B * S                  # 4096 tokens total
N_CORES = 8
OPC = D // N_CORES         # 128 out dims per core
HPC = H // N_CORES         # 2 heads per core
NI = D // 128              # 8 contraction chunks of 128
TCH = 512                  # projection token chunk
NTCH = T // TCH            # 8
QCH = 512                  # attention q chunk
NQCH = S // QCH            # 4 per batch
NKT = S // 128             # 16 key tiles per batch
NG = NKT // 2              # 8 score groups (2 k-tiles each) per q chunk
HW = DH + 2                # 66 cols per head in the v tile (data|ones|pad)
VW = HPC * HW              # 132

F32 = mybir.dt.float32
F32R = mybir.dt.float32r
BF16 = mybir.dt.bfloat16
EXP = mybir.ActivationFunctionType.Exp
MM_NP = ml_dtypes.bfloat16


def _mha_kernel(tc, y, xT, wq, wk, wv, woT, bq, bk, bv):
    with ExitStack() as ctx:
        _mha_kernel_inner(ctx, tc, y, xT, wq, wk, wv, woT, bq, bk, bv)


def _mha_kernel_inner(ctx: ExitStack, tc, y, xT, wq, wk, wv, woT, bq, bk, bv):
    nc = tc.nc
    pers = ctx.enter_context(tc.tile_pool(name="pers", bufs=1))

    qT = pers.tile([128, T], BF16, tag="qT")
    kT = pers.tile([128, T], BF16, tag="kT")
    vtk = pers.tile([128, B * NKT, VW], F32R, tag="vtk")
    xfull = pers.tile([128, NI, T], BF16, tag="xfull")
    wq_sb = pers.tile([128, NI, OPC], BF16, tag="wq")
    wk_sb = pers.tile([128, NI, OPC], BF16, tag="wk")
    wv_sb = pers.tile([128, NI, OPC], BF16, tag="wv")
    woT_sb = pers.tile([128, D], BF16, tag="wo")
    bq_sb = pers.tile([128, 1], F32, tag="bq")
    bk_sb = pers.tile([128, 1], F32, tag="bk")
    bv_sb = pers.tile([128, 1], F32, tag="bv")
    ident = pers.tile([128, 128], F32, tag="ident")

    # identity build first: it shares the gpsimd queue with the weight
    # DMAs and everything PE-transpose waits on it
    make_identity(nc, ident)
    # one DMA per weight tensor (dma_start costs ~0.6us of issue time on
    # the queueing engine; 28 per-chunk DMAs serialized the old lead-in)
    nc.gpsimd.dma_start(wk_sb[:, :, :], wk[:, :, :])
    nc.gpsimd.dma_start(wv_sb[:, :, :], wv[:, :, :])
    nc.gpsimd.dma_start(wq_sb[:, :, :], wq[:, :, :])
    nc.gpsimd.dma_start(woT_sb, woT)
    nc.gpsimd.dma_start(bk_sb, bk)
    nc.gpsimd.dma_start(bv_sb, bv)
    nc.gpsimd.dma_start(bq_sb, bq)
    # x: one token-major DMA per 512-token chunk (all 8 contraction
    # slices in one InstDMACopy, which fans out over the 16 SDMA slots)
    for t in range(NTCH):
        src_ap = bass.AP(
            tensor=xT.tensor,
            offset=xT.offset + t * TCH,
            ap=[[T, 128], [128 * T, NI], [1, TCH]],
        )
        nc.sync.dma_start(xfull[:, :, t * TCH : (t + 1) * TCH], src_ap)
    # constant ones/pad columns of vtk (broadcast along the k-tile dim)
    onepad = pers.tile([128, 2], F32, tag="onepad")
    nc.vector.memset(onepad[:, 0:1], 1.0)
    nc.vector.memset(onepad[:, 1:2], 0.0)
    onepad_b = bass.AP(
        tensor=onepad.tensor,
        offset=onepad.offset,
        ap=[onepad.ap[0], [0, B * NKT], onepad.ap[1]],
    )
    for h in range(HPC):
        nc.vector.tensor_copy(
            vtk[:, :, h * HW + DH : h * HW + DH + 2], onepad_b
        )

    # PSUM budget (8 banks):
    #   ps_s  [128, 2x512] f32  2 banks, bufs=2 (score tiles)
    #   pvout [66, 512]    f32  1 bank,  bufs=2 (PV accumulators)
    #   gen   [128, 512]   f32  1 bank,  bufs=1 (projections, transposes)
    #   out   [128, 512]   f32  1 bank,  bufs=1 (output projection)
    psum = ctx.enter_context(tc.tile_pool(name="psum", bufs=1, space="PSUM"))
    vstage = ctx.enter_context(tc.tile_pool(name="vstage", bufs=2))
    att = ctx.enter_context(tc.tile_pool(name="att", bufs=6))
    sm = ctx.enter_context(tc.tile_pool(name="sm", bufs=3))
    yo_p = ctx.enter_context(tc.tile_pool(name="yo_p", bufs=4))

    def proj_mm(dst, w_sb, b_sb, t):
        """PE thunks for one projection over token chunk t: 8
        accumulating matmuls into the gen psum slot, then a DVE bias-add
        drain into dst (a [128, TCH] bf16 AP)."""
        ps = psum.tile([128, TCH], F32, tag="gen", bufs=1, name=f"ps_{t}")

        def mk(i):
            def f():
                nc.tensor.matmul(
                    ps,
                    w_sb[:, i, :],
                    xfull[:, i, t * TCH : (t + 1) * TCH],
                    start=(i == 0),
                    stop=(i == NI - 1),
                )
                if i == NI - 1:
                    nc.vector.tensor_scalar_add(dst, ps, b_sb)

            return f

        for i in range(NI):
            yield mk(i)

    def v_chunk(t):
        """PE thunks: V projection for chunk t, then transposes into
        vtk's token-major k-tiles."""
        vs = vstage.tile([128, TCH], F32, tag="vs", name=f"vs_{t}")
        yield from proj_mm(vs[:, :], wv_sb, bv_sb, t)

        def mk_tr(j):
            def f():
                g = (t * TCH) // 128 + j
                ps_t = psum.tile([128, 128], F32, tag="gen", bufs=1,
                                 name=f"ps_t_{t}_{j}")
                nc.tensor.transpose(ps_t, vs[:, j * 128 : (j + 1) * 128], ident)
                # both heads' 64-col slices in one strided DVE copy
                dst = bass.AP(
                    tensor=vtk.tensor,
                    offset=vtk.offset + g * VW,
                    ap=[vtk.ap[0], [HW, HPC], [1, DH]],
                )
                src_ = bass.AP(
                    tensor=ps_t.tensor,
                    offset=ps_t.offset,
                    ap=[ps_t.ap[0], [DH, HPC], [1, DH]],
                )
                nc.vector.tensor_copy(dst, src_)
            return f

        for j in range(TCH // 128):
            yield mk_tr(j)

    # ---- work stream: all projection thunks, with named checkpoints ----
    # Order: k0 q0 | v0 k1 v1 k2 v2 k3 v3 | q1 k4 v4 q2 k5 v5 q3 k6 v6
    # k7 v7 | q4 q5 q6 q7.  Attention blocks force-drain to the
    # checkpoint they need (k-chunk per score tile, v-chunk per PV tile,
    # q-chunk per block) and otherwise pull opportunistically.
    stream = []
    cp = {}

    def add(name, gen):
        stream.extend(gen)
        cp[name] = len(stream)

    def proj_mm_sub(dst, w_sb, b_sb, t, c0, c1):
        ps = psum.tile([128, c1 - c0], F32, tag="gen", bufs=1,
                       name=f"ps_{t}_{c0}")

        def mk(i):
            def f():
                nc.tensor.matmul(
                    ps,
                    w_sb[:, i, :],
                    xfull[:, i, t * TCH + c0 : t * TCH + c1],
                    start=(i == 0),
                    stop=(i == NI - 1),
                )
                if i == NI - 1:
                    nc.vector.tensor_scalar_add(dst, ps, b_sb)

            return f

        for i in range(NI):
            yield mk(i)

    add("q0", proj_mm(qT[:, 0:TCH], wq_sb, bq_sb, 0))
    add("k0a", proj_mm_sub(kT[:, 0:128], wk_sb, bk_sb, 0, 0, 128))
    add("k0", proj_mm_sub(kT[:, 128:TCH], wk_sb, bk_sb, 0, 128, TCH))
    add("v0", v_chunk(0))
    for t in range(1, NQCH):
        add(f"k{t}", proj_mm(kT[:, t * TCH : (t + 1) * TCH], wk_sb, bk_sb, t))
        add(f"v{t}", v_chunk(t))
    # interleave remaining q chunks among b1 k/v so every checkpoint is
    # reached by gentle pacing instead of a burst:
    # q1 k4 v4 q2 k5 v5 q3 q4 k6 v6 q5 k7 q6 v7 q7
    _q = lambda t: add(f"q{t}",
                       proj_mm(qT[:, t * TCH : (t + 1) * TCH], wq_sb, bq_sb, t))
    _k = lambda t: add(f"k{t}",
                       proj_mm(kT[:, t * TCH : (t + 1) * TCH], wk_sb, bk_sb, t))
    _v = lambda t: add(f"v{t}", v_chunk(t))
    _q(1); _k(4); _v(4); _q(2); _k(5); _v(5); _q(3); _q(4)
    _k(6); _v(6); _q(5); _k(7); _q(6); _v(7); _q(7)

    pulled = [0]

    def pull_to(level):
        level = min(int(level), len(stream))
        while pulled[0] < level:
            stream[pulled[0]]()
            pulled[0] += 1

    def drain_to(name):
        pull_to(cp[name])

    def attention_block(b, qc, tail_thunks, lvl0, lvl1):
        dlvl = lvl1 - lvl0
        q0 = b * S + qc * QCH
        at_tiles = []
        pvs = [None, None]

        def scores_group(g2):
            if b == 0 and qc == 0 and g2 == 0:
                drain_to("k0a")
            else:
                drain_to(f"k{b * NQCH + g2 // 4}")
            ps_s = psum.tile([128, HPC * QCH], F32, tag="ps_s", bufs=2,
                             name=f"ps_s_{b}_{qc}_{g2}")
            g = b * NKT + g2
            for h in range(HPC):
                hs = slice(h * DH, (h + 1) * DH)
                nc.tensor.matmul(
                    ps_s[:, h * QCH : (h + 1) * QCH],
                    kT[hs, g * 128 : (g + 1) * 128],
                    qT[hs, q0 : q0 + QCH],
                    start=True,
                    stop=True,
                )
            at = att.tile([128, HPC * QCH], F32R, tag="at",
                          name=f"at_{b}_{qc}_{g2}")
            nc.scalar.activation(at, ps_s, EXP, scale=0.125)
            at_tiles.append(at)

        def pv_group(g2):
            drain_to(f"v{(b * NKT + g2) // 4}")
            g = b * NKT + g2
            for h in range(HPC):
                nc.tensor.matmul(
                    pvs[h],
                    vtk[:, g, h * HW : (h + 1) * HW],
                    at_tiles[g2][:, h * QCH : (h + 1) * QCH],
                    start=(g2 == 0),
                    stop=(g2 == NKT - 1),
                )

        drain_to(f"q{b * NQCH + qc}")
        scores_group(0)
        scores_group(1)
        scores_group(2)
        pvs[0] = psum.tile([HW, QCH], F32, tag="pvout", bufs=2,
                           name=f"pv0_{b}_{qc}")
        pvs[1] = psum.tile([HW, QCH], F32, tag="pvout", bufs=2,
                           name=f"pv1_{b}_{qc}")
        for g2 in range(3, NKT):
            pv_group(g2 - 3)
            if tail_thunks:
                tail_thunks.pop(0)()
            pull_to(lvl0 + dlvl * (g2 - 2) / (NKT - 2))
            scores_group(g2)
        pv_group(NKT - 3)
        for f in tail_thunks:
            f()
        tail_thunks.clear()
        pv_group(NKT - 2)
        pv_group(NKT - 1)

        # normalize: ctx rows for head h = pv[0:64] * recip(pv[64])
        ctx_sb = sm.tile([128, QCH], BF16, tag="ctx", name=f"ctx_{b}_{qc}")
        for h in range(HPC):
            rraw = sm.tile([1, QCH], F32, tag="rraw", name=f"rraw_{b}_{qc}_{h}")
            nc.vector.tensor_copy(rraw, pvs[h][DH : DH + 1, :])
            rrow = sm.tile([1, QCH], F32, tag="rrow", name=f"rrow_{b}_{qc}_{h}")
            nc.vector.reciprocal_approx_fast(rrow, rraw)
            nrm = sm.tile([DH, QCH], F32, tag="nrm", name=f"nrm_{b}_{qc}_{h}")
            nc.gpsimd.partition_broadcast(nrm, rrow)
            nc.vector.tensor_mul(
                ctx_sb[h * DH : (h + 1) * DH, :], pvs[h][0:DH, :], nrm
            )

        # deferred out-projection: 8 (matmul, psum-drain) thunks spread
        # across the next block, on a dedicated psum bank
        new_tail = []
        yos = [yo_p.tile([128, D], F32, tag="yo", name=f"yo_{b}_{qc}_{t4}")
               for t4 in range(QCH // 128)]

        def mk_out(t4, nch):
            def f():
                ps_o = psum.tile([128, 512], F32, tag="out", bufs=1,
                                 name=f"ps_o_{b}_{qc}_{t4}_{nch}")
                nc.tensor.matmul(
                    ps_o,
                    ctx_sb[:, t4 * 128 : (t4 + 1) * 128],
                    woT_sb[:, nch * 512 : (nch + 1) * 512],
                    start=True,
                    stop=True,
                )
                nc.vector.tensor_copy(
                    yos[t4][:, nch * 512 : (nch + 1) * 512], ps_o)
                if nch == 1:
                    r0 = q0 + t4 * 128
                    nc.gpsimd.dma_start(y[r0 : r0 + 128, :], yos[t4])
            return f

        for t4 in range(QCH // 128):
            for nch in range(D // 512):
                new_tail.append(mk_out(t4, nch))
        return new_tail

    # ---- emission ----
    # linear drain schedule: block (0,0) force-drains its own needs
    # (~88 thunks); the remaining stream is spread evenly over the rest
    tail_thunks = []
    n_blocks = B * NQCH
    if os.environ.get("MHA_SERIAL"):
        pull_to(10**9)
    base = cp["v3"]
    rest = len(stream) - base
    # end-of-block targets: everything the NEXT block needs at its start
    # (its q chunk), or the linear share, whichever is larger
    targets = []
    for bi in range(n_blocks):
        lin = base + rest * bi / (n_blocks - 1)
        nxt = cp[f"q{bi + 1}"] if bi + 1 < n_blocks else len(stream)
        targets.append(max(lin, nxt))
    for bi in range(n_blocks):
        b, qc = divmod(bi, NQCH)
        lvl0 = base if bi == 0 else targets[bi - 1]
        tail_thunks = attention_block(b, qc, tail_thunks, lvl0, targets[bi])
    pull_to(10**9)
    for f in tail_thunks:
        f()


_NC_CACHE = {}


def _build_nc(repeats=1):
    if repeats in _NC_CACHE:
        return _NC_CACHE[repeats]
    nc = bacc.Bacc("TRN2", target_bir_lowering=False, debug=False, num_devices=N_CORES)
    xT = nc.dram_tensor("xT", [NI, 128, T], BF16, kind="ExternalInput").ap()
    wq = nc.dram_tensor("wq", [128, NI, OPC], BF16, kind="ExternalInput").ap()
    wk = nc.dram_tensor("wk", [128, NI, OPC], BF16, kind="ExternalInput").ap()
    wv = nc.dram_tensor("wv", [128, NI, OPC], BF16, kind="ExternalInput").ap()
    woT = nc.dram_tensor("woT", [128, D], BF16, kind="ExternalInput").ap()
    bq = nc.dram_tensor("bq", [128, 1], F32, kind="ExternalInput").ap()
    bk = nc.dram_tensor("bk", [128, 1], F32, kind="ExternalInput").ap()
    bv = nc.dram_tensor("bv", [128, 1], F32, kind="ExternalInput").ap()
    y = nc.dram_tensor("y", [T, D], F32, kind="ExternalOutput").ap()
    with tile.TileContext(nc) as tc:
        for _ in range(repeats):
            _mha_kernel(tc, y, xT, wq, wk, wv, woT, bq, bk, bv)
    nc.compile()
    _NC_CACHE[repeats] = nc
    return nc


def _prep_in_maps(inputs):
    x = np.asarray(inputs["x"], np.float32)
    Wq = np.asarray(inputs["Wq"], np.float32)
    Wk = np.asarray(inputs["Wk"], np.float32)
    Wv = np.asarray(inputs["Wv"], np.float32)
    Wo = np.asarray(inputs["Wo"], np.float32)
    bq = np.asarray(inputs["bq"], np.float32)
    bk = np.asarray(inputs["bk"], np.float32)
    bv = np.asarray(inputs["bv"], np.float32)

    xT_np = np.ascontiguousarray(x.reshape(T, D).T).reshape(NI, 128, T).astype(MM_NP)

    def _w_slice(W, c):
        # [128(p), NI, OPC]: [p, i, o] = W[c*OPC+o, i*128+p]
        A = np.ascontiguousarray(W[c * OPC : (c + 1) * OPC, :].T)  # [D, OPC]
        return np.ascontiguousarray(A.reshape(NI, 128, OPC).transpose(1, 0, 2)).astype(
            MM_NP
        )

    in_maps = []
    for c in range(N_CORES):
        sl = slice(c * OPC, (c + 1) * OPC)
        in_maps.append(
            {
                "xT": xT_np,
                "wq": _w_slice(Wq, c),
                "wk": _w_slice(Wk, c),
                "wv": _w_slice(Wv, c),
                "woT": np.ascontiguousarray(Wo[:, sl].T).astype(MM_NP),
                "bq": bq[sl].reshape(OPC, 1).copy(),
                "bk": bk[sl].reshape(OPC, 1).copy(),
                "bv": bv[sl].reshape(OPC, 1).copy(),
            }
        )
    return in_maps


def kernel(**inputs) -> np.ndarray:
    nc = _build_nc()
    in_maps = _prep_in_maps(inputs)
    res = run_bass_kernel_spmd(nc, in_maps, core_ids=list(range(N_CORES)))
    bo = np.asarray(inputs["bo"], np.float32)
    y = np.zeros((T, D), np.float32)
    for c in range(N_CORES):
        y += res.results[c]["y"].astype(np.float32)
    y = (y + bo).astype(np.float32)
    return y.reshape(B, S, D)
